# revision 5
# baseline (speedup 1.0000x reference)
"""Trainium2 Bass kernel for nn_DynamicFeedForward (embedding-gather dot products).

Reference computation:
    part_weight = weight[mask]            # [b, s, 32, 512] gather
    out = einsum('bsh,bsmh->bsm', x, part_weight) + bias[mask]
    out = relu(out)

Strategy (data-parallel over tokens, 8 cores) — fp16 gather, 4 SWDGE rings:
  - 8192 tokens sharded 1024/core, groups of 128 (one token per partition).
  - Host compacts the table per core: the ~24K unique rows a core touches are
    written to a private [32768, 512] fp16 table; indices remapped to the
    compact id (always >= 0, so no int16 sign tricks / trailing-drop fixups).
  - Rows are fetched with GPSIMD dma_gather: 1024 rows x 1024B per call
    (half the bytes of fp32; tolerance 2e-2 >> fp16 dot error ~5e-4).
  - Bias is gathered on host ([tok, 32] fp32 input) and added on-device.
  - Gathers alternate across 4 SWDGE queues: each queue has its own Q7
      cpu-pair + descriptor ring, so ring drain (the ~8ns/desc wall seen
      with one queue) parallelizes.
  - Compute per chunk (128 tok x 8 cand x 512):
      one batched DVE tensor_tensor multiply (2x 16-bit packed mode),
      cands 0-3: DVE binary-tree adds at 2x + tensor_reduce, batched
        across the group's 4 chunks to amortize instruction overheads,
      cands 4-7: ACT Copy-activation accum_out (~830ns each).
  - Target: DMA-bound at ~33.5 MB/core over 4 rings.
"""

import numpy as np

N_CORES = 8
TOKENS = 4 * 2048
HIDDEN = 512
M = 32
VOCAB = 50000
P = 128
TOK_PER_CORE = TOKENS // N_CORES          # 1024
GROUPS = TOK_PER_CORE // P                # 8
M_TILE = 8                                # candidates per dma_gather chunk
NCHUNK = M // M_TILE                      # 4
WCAP = 32768                              # compacted-table capacity (>= unique)
NIDX = P * M_TILE                         # 1024 indices per gather (%128, <=1024)
N_DVE = 4                                 # candidates reduced on DVE (rest ACT)
N_DVE = 4                                 # candidates reduced on DVE (rest ACT)

_cached = None


def _build_program(repeats=1):
    import concourse.bacc as bacc
    import concourse.mybir as mybir
    import concourse.tile as tile

    f32 = mybir.dt.float32
    f16 = mybir.dt.float16
    i16 = mybir.dt.int16
    mult = mybir.AluOpType.mult
    add = mybir.AluOpType.add

    nc = bacc.Bacc(
        "TRN2",
        target_bir_lowering=False,
        debug=False,
        num_devices=N_CORES,
        num_swdge_queues=4,
    )

    x_d = nc.dram_tensor("x", [P, GROUPS * HIDDEN], f16, kind="ExternalInput")
    idx_d = nc.dram_tensor(
        "idx", [P, GROUPS * NCHUNK * (NIDX // 16)], i16, kind="ExternalInput"
    )
    w_d = nc.dram_tensor("w", [WCAP, HIDDEN], f16, kind="ExternalInput")
    bias_d = nc.dram_tensor("bias", [P, GROUPS * M], f32, kind="ExternalInput")
    out_d = nc.dram_tensor("out", [TOK_PER_CORE, M], f32, kind="ExternalOutput")

    with tile.TileContext(nc) as tc:
        with (
            tc.tile_pool(name="wg", bufs=5) as wpool,
            tc.tile_pool(name="xt", bufs=1) as xpool,
            tc.tile_pool(name="idxt", bufs=1) as ipool,
            tc.tile_pool(name="biast", bufs=1) as bpool,
            tc.tile_pool(name="prod", bufs=2) as ppool,
            tc.tile_pool(name="tree", bufs=2) as tpool,
            tc.tile_pool(name="rest", bufs=3) as rpool,
            tc.tile_pool(name="relut", bufs=3) as relupool,
            tc.tile_pool(name="dump", bufs=4) as dpool,
        ):
            # preload indices (4 KB/part), x shard (8 KB/part), biases (1 KB/part)
            it_all = ipool.tile([P, GROUPS * NCHUNK * (NIDX // 16)], i16)
            nc.sync.dma_start(it_all[:], idx_d[:, :])
            x_all = xpool.tile([P, GROUPS * HIDDEN], f16)
            nc.sync.dma_start(x_all[:], x_d[:, :])
            b_all = bpool.tile([P, GROUPS * M], f32)
            nc.sync.dma_start(b_all[:], bias_d[:, :])

            for g in [g for _ in range(repeats) for g in range(GROUPS)]:
                tok = slice(g * P, (g + 1) * P)
                x_t = x_all[:, g * HIDDEN : (g + 1) * HIDDEN]

                res_t = rpool.tile([P, M], f32)
                prod_g = ppool.tile([P, NCHUNK * M_TILE * HIDDEN], f16)
                pg4 = prod_g[:].rearrange("p (h c e) -> p h c e", c=M_TILE, e=HIDDEN)
                for h in range(NCHUNK):
                    k = g * NCHUNK + h
                    it = it_all[:, k * (NIDX // 16) : (k + 1) * (NIDX // 16)]

                    w_t = wpool.tile([P, M_TILE * HIDDEN], f16)
                    nc.gpsimd.dma_gather(
                        out_ap=w_t[:].rearrange("p (c e) -> p c e", e=HIDDEN),
                        in_ap=w_d[:, :],
                        idxs_ap=it,
                        num_idxs=NIDX,
                        num_idxs_reg=NIDX,
                        elem_size=HIDDEN,
                        queue_num=k % 4,
                    )
                    # one batched 2x multiply for the whole chunk
                    nc.vector.tensor_tensor(
                        out=pg4[:, h],
                        in0=w_t[:].rearrange("p (c e) -> p c e", e=HIDDEN),
                        in1=x_t[:, None, :].to_broadcast([P, M_TILE, HIDDEN]),
                        op=mult,
                    )
                    # cands N_DVE..7: ACT Copy-activation accumulate per chunk
                    for c in range(N_DVE, M_TILE):
                        dump = dpool.tile([P, HIDDEN], f16)
                        nc.scalar.activation(
                            out=dump[:],
                            in_=prod_g[
                                :,
                                (h * M_TILE + c) * HIDDEN : (h * M_TILE + c + 1)
                                * HIDDEN,
                            ],
                            func=mybir.ActivationFunctionType.Copy,
                            accum_out=res_t[
                                :, h * M_TILE + c : h * M_TILE + c + 1
                            ],
                        )

                # cands 0..N_DVE-1 of all 4 chunks: batched tree (2x) + reduce
                pd = pg4[:, :, 0:N_DVE, :]  # [P, NCHUNK, N_DVE, 512]
                t1 = tpool.tile([P, NCHUNK * N_DVE * 256], f16)
                t1v = t1[:].rearrange("p (h c e) -> p h c e", c=N_DVE, e=256)
                nc.vector.tensor_tensor(
                    out=t1v, in0=pd[:, :, :, 0:256], in1=pd[:, :, :, 256:512],
                    op=add,
                )
                t2 = tpool.tile([P, NCHUNK * N_DVE * 128], f16)
                t2v = t2[:].rearrange("p (h c e) -> p h c e", c=N_DVE, e=128)
                nc.vector.tensor_tensor(
                    out=t2v, in0=t1v[:, :, :, 0:128], in1=t1v[:, :, :, 128:256],
                    op=add,
                )
                t3 = tpool.tile([P, NCHUNK * N_DVE * 64], f16)
                t3v = t3[:].rearrange("p (h c e) -> p h c e", c=N_DVE, e=64)
                nc.vector.tensor_tensor(
                    out=t3v, in0=t2v[:, :, :, 0:64], in1=t2v[:, :, :, 64:128],
                    op=add,
                )
                t4 = tpool.tile([P, NCHUNK * N_DVE * 32], f16)
                t4v = t4[:].rearrange("p (h c e) -> p h c e", c=N_DVE, e=32)
                nc.vector.tensor_tensor(
                    out=t4v, in0=t3v[:, :, :, 0:32], in1=t3v[:, :, :, 32:64],
                    op=add,
                )
                # res columns for DVE cands: [h*8 + 0 .. h*8+N_DVE)
                nc.vector.tensor_reduce(
                    out=res_t[:].rearrange("p (h c) -> p h c", c=M_TILE)[
                        :, :, 0:N_DVE
                    ],
                    in_=t4v,
                    axis=mybir.AxisListType.X,
                    op=add,
                )
                # add host-gathered biases, relu, store
                sum_t = rpool.tile([P, M], f32)
                nc.vector.tensor_tensor(
                    out=sum_t[:],
                    in0=res_t[:],
                    in1=b_all[:, g * M : (g + 1) * M],
                    op=add,
                )
                relu_t = relupool.tile([P, M], f32)
                nc.scalar.activation(
                    relu_t[:], sum_t[:], mybir.ActivationFunctionType.Relu
                )
                nc.sync.dma_start(out_d[tok, :], relu_t[:])

    nc.compile()
    return nc


def _get_program():
    global _cached
    if _cached is None:
        _cached = _build_program()
    return _cached


def _pack_idx(remapped):
    """remapped: [TOK_PER_CORE, M] int16 compact indices (>= 0).

    Returns [P, GROUPS*NCHUNK*(NIDX//16)] int16 device layout: per (g, h)
    chunk the 1024-entry gather list (position i = cand-block i//128,
    token-partition i%128) wrapped into 16 partitions and replicated x8.
    """
    out = np.empty((GROUPS * NCHUNK, P, NIDX // 16), np.int16)
    for g in range(GROUPS):
        blk = remapped[g * P : (g + 1) * P]  # [128, M]
        for h in range(NCHUNK):
            lst = blk[:, h * M_TILE : (h + 1) * M_TILE].T.reshape(NIDX)
            wrapped = lst.reshape(NIDX // 16, 16).T  # [16, NIDX//16]
            out[g * NCHUNK + h] = np.tile(wrapped, (8, 1))
    return np.ascontiguousarray(out.transpose(1, 0, 2).reshape(P, -1))


def _prepare_in_maps(input_value, mask_tensor, weight, bias):
    x16 = np.asarray(input_value, np.float32).reshape(TOKENS, HIDDEN)
    x16 = x16.astype(np.float16)
    idx = np.asarray(mask_tensor).reshape(TOKENS, M).astype(np.int64)
    w16 = np.asarray(weight, np.float32).astype(np.float16)
    b32 = np.asarray(bias, np.float32)

    in_maps = []
    for c in range(N_CORES):
        t = slice(c * TOK_PER_CORE, (c + 1) * TOK_PER_CORE)
        idx_c = idx[t]
        u, inv = np.unique(idx_c.ravel(), return_inverse=True)
        assert len(u) <= WCAP
        remapped = inv.reshape(TOK_PER_CORE, M).astype(np.int16)
        table = np.empty((WCAP, HIDDEN), np.float16)
        table[: len(u)] = w16[u]
        bias_g = b32[idx_c]  # [TOK_PER_CORE, M] fp32
        bias_pm = np.ascontiguousarray(
            bias_g.reshape(GROUPS, P, M).transpose(1, 0, 2).reshape(P, -1)
        )
        x_pm = np.ascontiguousarray(
            x16[t].reshape(GROUPS, P, HIDDEN).transpose(1, 0, 2).reshape(P, -1)
        )
        in_maps.append(
            {"x": x_pm, "idx": _pack_idx(remapped), "w": table, "bias": bias_pm}
        )
    return in_maps


def kernel(input_value, mask_tensor, weight, bias):
    from concourse.bass_utils import run_bass_kernel_spmd

    nc = _get_program()
    in_maps = _prepare_in_maps(input_value, mask_tensor, weight, bias)
    res = run_bass_kernel_spmd(nc, in_maps, core_ids=list(range(N_CORES)))
    kernel._last_results = res

    out = np.concatenate(
        [np.array(res.results[c]["out"]) for c in range(N_CORES)], axis=0
    )
    return out.reshape(np.asarray(mask_tensor).shape).astype(np.float32)
